# revision 45
# baseline (speedup 1.0000x reference)
"""Trainium2 Bass kernel for nn_Cache_83897891160781 (topk_masking).

reference semantics:
  query (16,1,16,512) f32, keys (256,16,32768) f32
  q = query -> (T,B,H); pooling_keys = keys -> (B,N,L,H)
  scores[t,j,b,n] = q[t,b,:]. pooling_keys[b,n,j,:]
  scores = max over j (L=64)            -> (T,B,N)
  attention = softmax(scores/sqrt(H), axis=N)
  topk_indices = top_k(attention, 8)    -> (8,T,B) int32
  returns (attention, topk_indices)

Sharding: batch dim (B=16) across 8 cores, 2 batches/core (data parallel).

Per-core device kernel (SPMD, same program on all 8 cores):
  for each local batch b, for each group g of 8 key-blocks (512 positions):
    - one 1 MB DMA of host-relayouted (32x32-block-swapped) keys -> SBUF
    - one DVE stream-transpose (in-place 32x32 blocks) -> exact K^T in SBUF
      (bit-exact data movement; keeps the PE free for the matmuls)
    - 4 accumulating exact-fp32 matmuls (qT chunk stationary, K^T moving,
      N=512; fp32 rather than f32r/TF32 because TF32-level score noise
      demonstrably breaks top-k ordering on this data)
    - DVE max-reduce over 64-position blocks -> blockmax (16, 8) slice
  then per batch: softmax over N=256 (DVE max, ACT fused exp+sum, DVE
  reciprocal+scale) and native top-8 (nc.vector.max / max_index).

Measured ~210-230 us/core on HW (memory roofline ~178 us; the exact-fp32
PE matmul floor alone is ~219 us -- the kernel is PE-bound at that floor).
"""

import os
from contextlib import ExitStack

import numpy as np

T = 16          # queries
B = 16          # batch
H = 512         # hidden
L = 64          # tokens per cache block
N = 256         # cache blocks
TOPK = 8
NCORES = 8
BLOC = B // NCORES          # batches per core = 2
NGROUP = 32                 # position groups per batch
NPG = 8                     # blocks per group (8*64 = 512 positions)
SCALE = float(1.0 / np.sqrt(np.float32(512.0)))

_PROG = None


def _build_program(repeat=None):
    """Build the SPMD per-core program. With repeat=R, the whole compute is
    wrapped in a For_i(0, R) hardware loop (used only for timing)."""
    import concourse.mybir as mybir
    import concourse.tile as tile
    from concourse import bacc

    f32 = mybir.dt.float32
    f16 = mybir.dt.float16
    nc = bacc.Bacc("TRN2", target_bir_lowering=False, debug=False,
                   num_devices=NCORES)
    # keys pre-laid-out on host as 32x32-block-swapped tiles (see
    # _make_in_map), split into fp16 hi/lo planes (dim0: 0=hi, 1=lo) so the
    # score matmuls run at 1 cyc/col instead of fp32's 4. q.k is computed as
    # qh.kh + qh.kl + ql.kh; the dropped ql.kl term and the fp16 rounding of
    # the lo planes are ~2^-22 relative -- below fp32 accumulation noise.
    keys_d = nc.dram_tensor("keys", [2, BLOC, N // 2, 128, H], f16,
                            kind="ExternalInput")
    # qt pre-transposed on host: [hi/lo, k(128), b*64 + c*16 + t]
    qt_d = nc.dram_tensor("qt", [2, 128, BLOC * 4 * T], f16,
                          kind="ExternalInput")
    att_d = nc.dram_tensor("att", [BLOC, T, N], f32, kind="ExternalOutput")
    idx_d = nc.dram_tensor("idx", [BLOC, T, TOPK], mybir.dt.uint32,
                           kind="ExternalOutput")

    with ExitStack() as ctx:
        tc = ctx.enter_context(tile.TileContext(nc))
        const_pool = ctx.enter_context(tc.tile_pool(name="const", bufs=1))
        kin_pool = ctx.enter_context(tc.tile_pool(name="kin", bufs=6))
        kt_pool = ctx.enter_context(tc.tile_pool(name="kt", bufs=6))
        ps_pool = ctx.enter_context(tc.tile_pool(name="ps", bufs=6, space="PSUM"))
        sm_pool = ctx.enter_context(tc.tile_pool(name="sm", bufs=2))

        # qT, stationary operand: partition = h within chunk, free = (b,c,t)
        qt_h = const_pool.tile([128, BLOC * 4 * T], f16)
        nc.sync.dma_start(out=qt_h[:], in_=qt_d[0])
        qt_l = const_pool.tile([128, BLOC * 4 * T], f16)
        nc.sync.dma_start(out=qt_l[:], in_=qt_d[1])

        bms = [const_pool.tile([T, N], f32, name=f"bm{b}", tag=f"bm{b}")
               for b in range(BLOC)]

        if repeat is not None:
            ctx.enter_context(tc.For_i(
                0, repeat, 1,
                hint_engines=(mybir.EngineType.PE, mybir.EngineType.DVE,
                              mybir.EngineType.SP, mybir.EngineType.Activation),
            ))

        for b in range(BLOC):
            bm = bms[b]
            for g in range(NGROUP):
                # kts[k, j, c*128+w] = K[pos = j*128+w, h = c*128+k]
                kth = kt_pool.tile([128, 4, 512], f16, name="kth", tag="kth")
                ktl = kt_pool.tile([128, 4, 512], f16, name="ktl", tag="ktl")
                for s, kt_s in ((0, kth), (1, ktl)):
                    kin = kin_pool.tile([128, 2048], f16, name=f"kin{s}",
                                        tag=f"kin{s}")
                    nc.sync.dma_start(
                        out=kin[:],
                        in_=keys_d[s, b, 4 * g:4 * (g + 1)].rearrange(
                            "j p h -> p j h"),
                    )
                    # 32x32 in-place block transpose of the host-block-swapped
                    # tiles = exact K^T for all 4 j / 4 h-chunks (DVE 2x-mode
                    # data movement; frees the PE for the matmuls).
                    nc.vector.transpose(
                        kt_s[:].rearrange("p j f -> p (j f)"), kin[:])
                # scores (16 t, 512 pos): fp32-accurate via 3 fp16 planes per
                # h-chunk, accumulated in fp32 PSUM (12 matmuls).
                ps = ps_pool.tile([T, 512], f32)
                nmm = 0
                for c in range(4):
                    s = (b * 4 + c) * T
                    for qs, ks in ((qt_h, kth), (qt_h, ktl), (qt_l, kth)):
                        nc.tensor.matmul(
                            ps[:], lhsT=qs[:, s:s + T],
                            rhs=ks[:, :, 128 * c:128 * (c + 1)],
                            start=(nmm == 0), stop=(nmm == 11))
                        nmm += 1
                # max over each 64-position block
                nc.vector.reduce_max(
                    bm[:, g * NPG:(g + 1) * NPG],
                    ps[:].rearrange("t (n l) -> t n l", l=L),
                    mybir.AxisListType.X,
                )
            # ---- softmax over N=256 + top-8 for batch b ----
            mx = sm_pool.tile([T, 1], f32)
            nc.vector.reduce_max(mx[:], bm[:], mybir.AxisListType.X)
            nb = sm_pool.tile([T, 1], f32)
            nc.vector.tensor_scalar_mul(nb[:], mx[:], -SCALE)
            prob = sm_pool.tile([T, N], f32)
            se = sm_pool.tile([T, 1], f32)
            nc.scalar.activation(prob[:], bm[:],
                                 mybir.ActivationFunctionType.Exp,
                                 bias=nb[:], scale=SCALE, accum_out=se[:])
            ri = sm_pool.tile([T, 1], f32)
            nc.vector.reciprocal(ri[:], se[:])
            att_sb = sm_pool.tile([T, N], f32)
            nc.vector.tensor_scalar_mul(att_sb[:], prob[:], ri[:])
            m8 = sm_pool.tile([T, TOPK], f32)
            nc.vector.max(out=m8[:], in_=att_sb[:])
            ix = sm_pool.tile([T, TOPK], mybir.dt.uint32)
            nc.vector.max_index(ix[:], m8[:], att_sb[:])
            nc.sync.dma_start(att_d[b], att_sb[:])
            nc.sync.dma_start(idx_d[b], ix[:])
    nc.compile()
    return nc


def _get_prog():
    global _PROG
    if _PROG is None:
        _PROG = _build_program()
    return _PROG


# test.py can set this to collect profiling info; the last BassKernelResults
# is stashed in LAST_RESULT.
TRACE = bool(int(os.environ.get("BASS_KERNEL_TRACE", "0")))
LAST_RESULT = None


def _make_in_map(query, keys, i):
    """Host-side shard + relayout for core i (batches 2i, 2i+1)."""
    # keys (N, B, L*H) -> [b, m, p, h]: m = n//2, p = (n%2)*64 + pos
    kb = keys[:, i * BLOC:(i + 1) * BLOC, :]             # (N, 2, L*H)
    kb = kb.reshape(N // 2, 2, BLOC, L, H)               # (m, a, b, l, h)
    kb = np.transpose(kb, (2, 0, 1, 3, 4)).reshape(BLOC, N // 2, 128, H)
    # 32x32 block swap within each (128 pos, 128 h-chunk) matrix so the
    # device-side DVE stream-transpose (in-place 32x32 blocks) yields K^T:
    # X[b, m, 32u+i, 128c+32v+j] = kb[b, m, 32v+i, 128c+32u+j]
    v = kb.reshape(BLOC, N // 2, 4, 32, 4, 4, 32)        # b m U i C V j
    x = np.ascontiguousarray(np.transpose(v, (0, 1, 5, 3, 4, 2, 6)))
    x = x.reshape(BLOC, N // 2, 128, H)
    # fp16 hi/lo split: x == xh + xl to ~2^-22 relative
    xh = x.astype(np.float16)
    xl = (x - xh.astype(np.float32)).astype(np.float16)
    xs = np.stack([xh, xl])                              # (2, b, m, 128, H)
    # qt [k, b*64 + c*16 + t] = query[t, 0, 2i+b, c*128+k]
    q2 = query[:, 0, i * BLOC:(i + 1) * BLOC, :]         # (T, 2, H)
    qt = np.ascontiguousarray(
        np.transpose(q2.reshape(T, BLOC, 4, 128), (3, 1, 2, 0))
    ).reshape(128, BLOC * 4 * T)
    qh = qt.astype(np.float16)
    ql = (qt - qh.astype(np.float32)).astype(np.float16)
    return {"keys": xs, "qt": np.stack([qh, ql])}


def _make_runner(nc, in_maps):
    """Build a reusable jitted 8-core callable for nc (timing use)."""
    import jax
    import numpy as np_
    from jax.sharding import Mesh, PartitionSpec
    from jax.experimental.shard_map import shard_map
    import concourse.mybir as mybir
    from concourse import bass2jax

    bass2jax.install_neuronx_cc_hook()
    partition_name = (nc.partition_id_tensor.name
                      if nc.partition_id_tensor else None)
    in_names, out_names, out_avals, zero_outs = [], [], [], []
    for alloc in nc.m.functions[0].allocations:
        if not isinstance(alloc, mybir.MemoryLocationSet):
            continue
        name = alloc.memorylocations[0].name
        if alloc.kind == "ExternalInput":
            if name != partition_name:
                in_names.append(name)
        elif alloc.kind == "ExternalOutput":
            shape = tuple(alloc.tensor_shape)
            dtype = mybir.dt.np(alloc.dtype)
            out_names.append(name)
            out_avals.append(jax.core.ShapedArray(shape, dtype))
            zero_outs.append(np_.zeros(shape, dtype))
    n_params = len(in_names)
    all_names = in_names + out_names
    if partition_name is not None:
        all_names = all_names + [partition_name]

    def _body(*args):
        operands = list(args)
        if partition_name is not None:
            operands.append(bass2jax.partition_id_tensor())
        outs = bass2jax._bass_exec_p.bind(
            *operands, out_avals=tuple(out_avals), in_names=tuple(all_names),
            out_names=tuple(out_names), lowering_input_output_aliases=(),
            sim_require_finite=True, sim_require_nnan=True, nc=nc)
        return tuple(outs)

    n = len(in_maps)
    devices = jax.devices()[:n]
    mesh = Mesh(np.asarray(devices), ("core",))
    nspec = (PartitionSpec("core"),) * (n_params + len(out_names))
    jitted = jax.jit(shard_map(_body, mesh=mesh, in_specs=nspec,
                               out_specs=(PartitionSpec("core"),) * len(out_names),
                               check_rep=False), keep_unused=True)
    from jax.sharding import NamedSharding
    shard = NamedSharding(mesh, PartitionSpec("core"))
    # upload once; reuse device-resident arrays across timed calls
    concat_in = [jax.device_put(
        np.concatenate([np.asarray(m[nm]) for m in in_maps], axis=0), shard)
        for nm in in_names]
    concat_zeros = [jax.device_put(
        np.zeros((n * z.shape[0], *z.shape[1:]), z.dtype), shard)
        for z in zero_outs]
    jax.block_until_ready(concat_in)

    def run():
        outs = jitted(*concat_in, *concat_zeros)
        jax.block_until_ready(outs)
        return outs

    return run


def benchmark(query, keys, r_short=9, r_long=65, iters=12):
    """Return estimated per-iteration HW time (ns) via loop-delta timing:
    the same program is run with a For_i repeat of r_short vs r_long; the
    wall-time delta divided by the iteration delta cancels all per-call
    dispatch/load overhead."""
    import time
    query = np.asarray(query, dtype=np.float32)
    keys = np.asarray(keys, dtype=np.float32)
    in_maps = [_make_in_map(query, keys, i) for i in range(NCORES)]
    results = {}
    for r in (r_short, r_long):
        nc = _build_program(repeat=r)
        run = _make_runner(nc, in_maps)
        run()  # compile + warmup
        times = []
        for _ in range(iters):
            t0 = time.perf_counter()
            run()
            times.append(time.perf_counter() - t0)
        times.sort()
        best = times[0]
        med = times[len(times) // 2]
        results[r] = (best, med)
        print(f"  repeat={r}: best {best*1e3:.3f} ms  median {med*1e3:.3f} ms")
    ns_best = (results[r_long][0] - results[r_short][0]) / (r_long - r_short) * 1e9
    ns_med = (results[r_long][1] - results[r_short][1]) / (r_long - r_short) * 1e9
    print(f"  per-iteration: best-delta {ns_best:.0f} ns  median-delta {ns_med:.0f} ns")
    return ns_med


def kernel(query, keys):
    global LAST_RESULT
    from concourse.bass_utils import run_bass_kernel_spmd

    query = np.asarray(query, dtype=np.float32)
    keys = np.asarray(keys, dtype=np.float32)
    assert query.shape == (T, 1, B, H), query.shape
    assert keys.shape == (N, B, L * H), keys.shape

    nc = _get_prog()
    in_maps = []
    for i in range(NCORES):
        in_maps.append(_make_in_map(query, keys, i))

    res = run_bass_kernel_spmd(nc, in_maps, core_ids=list(range(NCORES)),
                               trace=TRACE)
    LAST_RESULT = res

    attention = np.empty((T, B, N), np.float32)
    topk = np.empty((TOPK, T, B), np.int32)
    for i, r in enumerate(res.results):
        attention[:, i * BLOC:(i + 1) * BLOC, :] = r["att"].transpose(1, 0, 2)
        topk[:, :, i * BLOC:(i + 1) * BLOC] = \
            r["idx"].astype(np.int32).transpose(2, 1, 0)
    return attention, topk


# revision 47
# speedup vs baseline: 1.1841x; 1.1841x over previous
"""Trainium2 Bass kernel for nn_Cache_83897891160781 (topk_masking).

reference semantics:
  query (16,1,16,512) f32, keys (256,16,32768) f32
  q = query -> (T,B,H); pooling_keys = keys -> (B,N,L,H)
  scores[t,j,b,n] = q[t,b,:]. pooling_keys[b,n,j,:]
  scores = max over j (L=64)            -> (T,B,N)
  attention = softmax(scores/sqrt(H), axis=N)
  topk_indices = top_k(attention, 8)    -> (8,T,B) int32
  returns (attention, topk_indices)

Sharding: batch dim (B=16) across 8 cores, 2 batches/core (data parallel).

Per-core device kernel (SPMD, same program on all 8 cores):
  for each local batch b, for each group g of 8 key-blocks (512 positions):
    - one 1 MB DMA of host-relayouted (32x32-block-swapped) keys -> SBUF
    - one DVE stream-transpose (in-place 32x32 blocks) -> exact K^T in SBUF
      (bit-exact data movement; keeps the PE free for the matmuls)
    - 4 accumulating exact-fp32 matmuls (qT chunk stationary, K^T moving,
      N=512; fp32 rather than f32r/TF32 because TF32-level score noise
      demonstrably breaks top-k ordering on this data)
    - DVE max-reduce over 64-position blocks -> blockmax (16, 8) slice
  then per batch: softmax over N=256 (DVE max, ACT fused exp+sum, DVE
  reciprocal+scale) and native top-8 (nc.vector.max / max_index).

Measured ~210-230 us/core on HW (memory roofline ~178 us; the exact-fp32
PE matmul floor alone is ~219 us -- the kernel is PE-bound at that floor).
"""

import os
from contextlib import ExitStack

import numpy as np

T = 16          # queries
B = 16          # batch
H = 512         # hidden
L = 64          # tokens per cache block
N = 256         # cache blocks
TOPK = 8
NCORES = 8
BLOC = B // NCORES          # batches per core = 2
NGROUP = 32                 # position groups per batch
NPG = 8                     # blocks per group (8*64 = 512 positions)
SCALE = float(1.0 / np.sqrt(np.float32(512.0)))

_PROG = None


def _build_program(repeat=None):
    """Build the SPMD per-core program. With repeat=R, the whole compute is
    wrapped in a For_i(0, R) hardware loop (used only for timing)."""
    import concourse.mybir as mybir
    import concourse.tile as tile
    from concourse import bacc

    f32 = mybir.dt.float32
    nc = bacc.Bacc("TRN2", target_bir_lowering=False, debug=False,
                   num_devices=NCORES)
    # keys pre-laid-out on host as 32x32-block-swapped tiles: for each
    # (block pair m, h-chunk c), the 128x128 matrix X satisfies
    # X[32u+i, 128c+32v+j] = K[pos=32v+i, h=128c+32u+j], so a DVE
    # stream-transpose (which transposes each 32x32 block in place) yields
    # K^T exactly -- no PE transposes, no PSUM staging, bit-exact.
    keys_d = nc.dram_tensor("keys", [BLOC, N // 2, 128, H], f32,
                            kind="ExternalInput")
    # qt pre-transposed on host: [k(128), b*64 + c*16 + t]
    qt_d = nc.dram_tensor("qt", [128, BLOC * 4 * T], f32,
                          kind="ExternalInput")
    att_d = nc.dram_tensor("att", [BLOC, T, N], f32, kind="ExternalOutput")
    idx_d = nc.dram_tensor("idx", [BLOC, T, TOPK], mybir.dt.uint32,
                           kind="ExternalOutput")

    with ExitStack() as ctx:
        tc = ctx.enter_context(tile.TileContext(nc))
        const_pool = ctx.enter_context(tc.tile_pool(name="const", bufs=1))
        kin_pool = ctx.enter_context(tc.tile_pool(name="kin", bufs=8))
        kt_pool = ctx.enter_context(tc.tile_pool(name="kt", bufs=6))
        ps_pool = ctx.enter_context(tc.tile_pool(name="ps", bufs=8, space="PSUM"))
        sm_pool = ctx.enter_context(tc.tile_pool(name="sm", bufs=2))

        # qT, stationary operand: partition = h within chunk, free = (b,c,t)
        qt_sb = const_pool.tile([128, BLOC * 4 * T], f32)
        nc.sync.dma_start(out=qt_sb[:], in_=qt_d[:])

        bms = [const_pool.tile([T, N], f32, name=f"bm{b}", tag=f"bm{b}")
               for b in range(BLOC)]

        if repeat is not None:
            ctx.enter_context(tc.For_i(
                0, repeat, 1,
                hint_engines=(mybir.EngineType.PE, mybir.EngineType.DVE,
                              mybir.EngineType.SP, mybir.EngineType.Activation),
            ))

        for b in range(BLOC):
            bm = bms[b]
            for g in range(NGROUP):
                # kts[k, j, c*128+w] = K[pos = j*128+w, h = c*128+k]
                kts = kt_pool.tile([128, 4, 512], f32)
                kin = kin_pool.tile([128, 2048], f32)
                nc.sync.dma_start(
                    out=kin[:],
                    in_=keys_d[b, 4 * g:4 * (g + 1)].rearrange("j p h -> p j h"),
                )
                # 32x32 in-place block transpose of the host-block-swapped
                # tiles = exact K^T for all 4 j / 4 h-chunks (DVE, bit-exact
                # data movement; frees the PE for the fp32 matmuls).
                nc.vector.transpose(kts[:].rearrange("p j f -> p (j f)"), kin[:])
                # scores (16 t, 512 pos) accumulated over 4 h-chunks,
                # in exact fp32 (TF32/f32r would break top-k ordering).
                ps = ps_pool.tile([T, 512], f32)
                for c in range(4):
                    s = (b * 4 + c) * T
                    nc.tensor.matmul(ps[:], lhsT=qt_sb[:, s:s + T],
                                     rhs=kts[:, :, 128 * c:128 * (c + 1)],
                                     start=(c == 0), stop=(c == 3))
                # max over each 64-position block
                nc.vector.reduce_max(
                    bm[:, g * NPG:(g + 1) * NPG],
                    ps[:].rearrange("t (n l) -> t n l", l=L),
                    mybir.AxisListType.X,
                )
            # ---- softmax over N=256 + top-8 for batch b ----
            mx = sm_pool.tile([T, 1], f32)
            nc.vector.reduce_max(mx[:], bm[:], mybir.AxisListType.X)
            nb = sm_pool.tile([T, 1], f32)
            nc.vector.tensor_scalar_mul(nb[:], mx[:], -SCALE)
            prob = sm_pool.tile([T, N], f32)
            se = sm_pool.tile([T, 1], f32)
            nc.scalar.activation(prob[:], bm[:],
                                 mybir.ActivationFunctionType.Exp,
                                 bias=nb[:], scale=SCALE, accum_out=se[:])
            ri = sm_pool.tile([T, 1], f32)
            nc.vector.reciprocal(ri[:], se[:])
            att_sb = sm_pool.tile([T, N], f32)
            nc.vector.tensor_scalar_mul(att_sb[:], prob[:], ri[:])
            m8 = sm_pool.tile([T, TOPK], f32)
            nc.vector.max(out=m8[:], in_=att_sb[:])
            ix = sm_pool.tile([T, TOPK], mybir.dt.uint32)
            nc.vector.max_index(ix[:], m8[:], att_sb[:])
            nc.sync.dma_start(att_d[b], att_sb[:])
            nc.sync.dma_start(idx_d[b], ix[:])
    nc.compile()
    return nc


def _get_prog():
    global _PROG
    if _PROG is None:
        _PROG = _build_program()
    return _PROG


# test.py can set this to collect profiling info; the last BassKernelResults
# is stashed in LAST_RESULT.
TRACE = bool(int(os.environ.get("BASS_KERNEL_TRACE", "0")))
LAST_RESULT = None


def _make_in_map(query, keys, i):
    """Host-side shard + relayout for core i (batches 2i, 2i+1)."""
    # keys (N, B, L*H) -> [b, m, p, h]: m = n//2, p = (n%2)*64 + pos
    kb = keys[:, i * BLOC:(i + 1) * BLOC, :]             # (N, 2, L*H)
    kb = kb.reshape(N // 2, 2, BLOC, L, H)               # (m, a, b, l, h)
    kb = np.transpose(kb, (2, 0, 1, 3, 4)).reshape(BLOC, N // 2, 128, H)
    # 32x32 block swap within each (128 pos, 128 h-chunk) matrix so the
    # device-side DVE stream-transpose (in-place 32x32 blocks) yields K^T:
    # X[b, m, 32u+i, 128c+32v+j] = kb[b, m, 32v+i, 128c+32u+j]
    v = kb.reshape(BLOC, N // 2, 4, 32, 4, 4, 32)        # b m U i C V j
    x = np.ascontiguousarray(np.transpose(v, (0, 1, 5, 3, 4, 2, 6)))
    x = x.reshape(BLOC, N // 2, 128, H)
    # qt [k, b*64 + c*16 + t] = query[t, 0, 2i+b, c*128+k]
    q2 = query[:, 0, i * BLOC:(i + 1) * BLOC, :]         # (T, 2, H)
    qt = np.ascontiguousarray(
        np.transpose(q2.reshape(T, BLOC, 4, 128), (3, 1, 2, 0))
    ).reshape(128, BLOC * 4 * T)
    return {"keys": x, "qt": qt}


def _make_runner(nc, in_maps):
    """Build a reusable jitted 8-core callable for nc (timing use)."""
    import jax
    import numpy as np_
    from jax.sharding import Mesh, PartitionSpec
    from jax.experimental.shard_map import shard_map
    import concourse.mybir as mybir
    from concourse import bass2jax

    bass2jax.install_neuronx_cc_hook()
    partition_name = (nc.partition_id_tensor.name
                      if nc.partition_id_tensor else None)
    in_names, out_names, out_avals, zero_outs = [], [], [], []
    for alloc in nc.m.functions[0].allocations:
        if not isinstance(alloc, mybir.MemoryLocationSet):
            continue
        name = alloc.memorylocations[0].name
        if alloc.kind == "ExternalInput":
            if name != partition_name:
                in_names.append(name)
        elif alloc.kind == "ExternalOutput":
            shape = tuple(alloc.tensor_shape)
            dtype = mybir.dt.np(alloc.dtype)
            out_names.append(name)
            out_avals.append(jax.core.ShapedArray(shape, dtype))
            zero_outs.append(np_.zeros(shape, dtype))
    n_params = len(in_names)
    all_names = in_names + out_names
    if partition_name is not None:
        all_names = all_names + [partition_name]

    def _body(*args):
        operands = list(args)
        if partition_name is not None:
            operands.append(bass2jax.partition_id_tensor())
        outs = bass2jax._bass_exec_p.bind(
            *operands, out_avals=tuple(out_avals), in_names=tuple(all_names),
            out_names=tuple(out_names), lowering_input_output_aliases=(),
            sim_require_finite=True, sim_require_nnan=True, nc=nc)
        return tuple(outs)

    n = len(in_maps)
    devices = jax.devices()[:n]
    mesh = Mesh(np.asarray(devices), ("core",))
    nspec = (PartitionSpec("core"),) * (n_params + len(out_names))
    jitted = jax.jit(shard_map(_body, mesh=mesh, in_specs=nspec,
                               out_specs=(PartitionSpec("core"),) * len(out_names),
                               check_rep=False), keep_unused=True)
    from jax.sharding import NamedSharding
    shard = NamedSharding(mesh, PartitionSpec("core"))
    # upload once; reuse device-resident arrays across timed calls
    concat_in = [jax.device_put(
        np.concatenate([np.asarray(m[nm]) for m in in_maps], axis=0), shard)
        for nm in in_names]
    concat_zeros = [jax.device_put(
        np.zeros((n * z.shape[0], *z.shape[1:]), z.dtype), shard)
        for z in zero_outs]
    jax.block_until_ready(concat_in)

    def run():
        outs = jitted(*concat_in, *concat_zeros)
        jax.block_until_ready(outs)
        return outs

    return run


def benchmark(query, keys, r_short=9, r_long=65, iters=12):
    """Return estimated per-iteration HW time (ns) via loop-delta timing:
    the same program is run with a For_i repeat of r_short vs r_long; the
    wall-time delta divided by the iteration delta cancels all per-call
    dispatch/load overhead."""
    import time
    query = np.asarray(query, dtype=np.float32)
    keys = np.asarray(keys, dtype=np.float32)
    in_maps = [_make_in_map(query, keys, i) for i in range(NCORES)]
    results = {}
    for r in (r_short, r_long):
        nc = _build_program(repeat=r)
        run = _make_runner(nc, in_maps)
        run()  # compile + warmup
        times = []
        for _ in range(iters):
            t0 = time.perf_counter()
            run()
            times.append(time.perf_counter() - t0)
        times.sort()
        best = times[0]
        med = times[len(times) // 2]
        results[r] = (best, med)
        print(f"  repeat={r}: best {best*1e3:.3f} ms  median {med*1e3:.3f} ms")
    ns_best = (results[r_long][0] - results[r_short][0]) / (r_long - r_short) * 1e9
    ns_med = (results[r_long][1] - results[r_short][1]) / (r_long - r_short) * 1e9
    print(f"  per-iteration: best-delta {ns_best:.0f} ns  median-delta {ns_med:.0f} ns")
    return ns_med


def kernel(query, keys):
    global LAST_RESULT
    from concourse.bass_utils import run_bass_kernel_spmd

    query = np.asarray(query, dtype=np.float32)
    keys = np.asarray(keys, dtype=np.float32)
    assert query.shape == (T, 1, B, H), query.shape
    assert keys.shape == (N, B, L * H), keys.shape

    nc = _get_prog()
    in_maps = []
    for i in range(NCORES):
        in_maps.append(_make_in_map(query, keys, i))

    res = run_bass_kernel_spmd(nc, in_maps, core_ids=list(range(NCORES)),
                               trace=TRACE)
    LAST_RESULT = res

    attention = np.empty((T, B, N), np.float32)
    topk = np.empty((TOPK, T, B), np.int32)
    for i, r in enumerate(res.results):
        attention[:, i * BLOC:(i + 1) * BLOC, :] = r["att"].transpose(1, 0, 2)
        topk[:, :, i * BLOC:(i + 1) * BLOC] = \
            r["idx"].astype(np.int32).transpose(2, 1, 0)
    return attention, topk


# revision 48
# speedup vs baseline: 1.5824x; 1.3364x over previous
"""Trainium2 Bass kernel for nn_Cache_83897891160781 (topk_masking).

reference semantics:
  query (16,1,16,512) f32, keys (256,16,32768) f32
  q = query -> (T,B,H); pooling_keys = keys -> (B,N,L,H)
  scores[t,j,b,n] = q[t,b,:]. pooling_keys[b,n,j,:]
  scores = max over j (L=64)            -> (T,B,N)
  attention = softmax(scores/sqrt(H), axis=N)
  topk_indices = top_k(attention, 8)    -> (8,T,B) int32
  returns (attention, topk_indices)

Sharding: batch dim (B=16) across 8 cores, 2 batches/core (data parallel).

Per-core device kernel (SPMD, same program on all 8 cores):
  for each local batch b, for each group g of 8 key-blocks (512 positions):
    - one 1 MB DMA of host-relayouted (32x32-block-swapped) keys -> SBUF
    - one DVE stream-transpose (in-place 32x32 blocks) -> exact K^T in SBUF
      (bit-exact data movement; keeps the PE free for the matmuls)
    - 4 accumulating exact-fp32 matmuls (qT chunk stationary, K^T moving,
      N=512; fp32 rather than f32r/TF32 because TF32-level score noise
      demonstrably breaks top-k ordering on this data)
    - DVE max-reduce over 64-position blocks -> blockmax (16, 8) slice
  then per batch: softmax over N=256 (DVE max, ACT fused exp+sum, DVE
  reciprocal+scale) and native top-8 (nc.vector.max / max_index).

Measured ~210-230 us/core on HW (memory roofline ~178 us; the exact-fp32
PE matmul floor alone is ~219 us -- the kernel is PE-bound at that floor).
"""

import os
from contextlib import ExitStack

import numpy as np

T = 16          # queries
B = 16          # batch
H = 512         # hidden
L = 64          # tokens per cache block
N = 256         # cache blocks
TOPK = 8
NCORES = 8
BLOC = B // NCORES          # batches per core = 2
NGROUP = 32                 # position groups per batch
NPG = 8                     # blocks per group (8*64 = 512 positions)
SCALE = float(1.0 / np.sqrt(np.float32(512.0)))

_PROG = None


def _build_program(repeat=None):
    """Build the SPMD per-core program. With repeat=R, the whole compute is
    wrapped in a For_i(0, R) hardware loop (used only for timing)."""
    import concourse.mybir as mybir
    import concourse.tile as tile
    from concourse import bacc

    f32 = mybir.dt.float32
    nc = bacc.Bacc("TRN2", target_bir_lowering=False, debug=False,
                   num_devices=NCORES)
    # keys pre-laid-out on host as 32x32-block-swapped tiles: for each
    # (block pair m, h-chunk c), the 128x128 matrix X satisfies
    # X[32u+i, 128c+32v+j] = K[pos=32v+i, h=128c+32u+j], so a DVE
    # stream-transpose (which transposes each 32x32 block in place) yields
    # K^T exactly -- no PE transposes, no PSUM staging, bit-exact.
    keys_d = nc.dram_tensor("keys", [BLOC, N // 2, 128, H], f32,
                            kind="ExternalInput")
    # qt pre-transposed on host: [k(128), b*64 + c*16 + t]
    qt_d = nc.dram_tensor("qt", [128, BLOC * 4 * T], f32,
                          kind="ExternalInput")
    att_d = nc.dram_tensor("att", [BLOC, T, N], f32, kind="ExternalOutput")
    idx_d = nc.dram_tensor("idx", [BLOC, T, TOPK], mybir.dt.uint32,
                           kind="ExternalOutput")

    with ExitStack() as ctx:
        tc = ctx.enter_context(tile.TileContext(nc))
        const_pool = ctx.enter_context(tc.tile_pool(name="const", bufs=1))
        kin_pool = ctx.enter_context(tc.tile_pool(name="kin", bufs=6))
        kt_pool = ctx.enter_context(tc.tile_pool(name="kt", bufs=6))
        ps_pool = ctx.enter_context(tc.tile_pool(name="ps", bufs=6, space="PSUM"))
        sm_pool = ctx.enter_context(tc.tile_pool(name="sm", bufs=2))

        # qT, stationary operand: partition = h within chunk, free = (b,c,t)
        qt_sb = const_pool.tile([128, BLOC * 4 * T], f32)
        nc.sync.dma_start(out=qt_sb[:], in_=qt_d[:])

        bms = [const_pool.tile([T, N], f32, name=f"bm{b}", tag=f"bm{b}")
               for b in range(BLOC)]

        if repeat is not None:
            ctx.enter_context(tc.For_i(
                0, repeat, 1,
                hint_engines=(mybir.EngineType.PE, mybir.EngineType.DVE,
                              mybir.EngineType.SP, mybir.EngineType.Activation),
            ))

        for b in range(BLOC):
            bm = bms[b]
            for g in range(NGROUP):
                # kts[k, j, c*128+w] = K[pos = j*128+w, h = c*128+k]
                kts = kt_pool.tile([128, 4, 512], f32)
                kin = kin_pool.tile([128, 2048], f32)
                nc.sync.dma_start(
                    out=kin[:],
                    in_=keys_d[b, 4 * g:4 * (g + 1)].rearrange("j p h -> p j h"),
                )
                # 32x32 in-place block transpose of the host-block-swapped
                # tiles = exact K^T for all 4 j / 4 h-chunks (DVE, bit-exact
                # data movement; frees the PE for the fp32 matmuls).
                nc.vector.transpose(kts[:].rearrange("p j f -> p (j f)"), kin[:])
                # scores (16 t, 512 pos) accumulated over 4 h-chunks,
                # in exact fp32 (TF32/f32r would break top-k ordering).
                ps = ps_pool.tile([T, 512], f32)
                for c in range(4):
                    s = (b * 4 + c) * T
                    nc.tensor.matmul(ps[:], lhsT=qt_sb[:, s:s + T],
                                     rhs=kts[:, :, 128 * c:128 * (c + 1)],
                                     start=(c == 0), stop=(c == 3))
                # max over each 64-position block
                nc.vector.reduce_max(
                    bm[:, g * NPG:(g + 1) * NPG],
                    ps[:].rearrange("t (n l) -> t n l", l=L),
                    mybir.AxisListType.X,
                )
            # ---- softmax over N=256 + top-8 for batch b ----
            mx = sm_pool.tile([T, 1], f32)
            nc.vector.reduce_max(mx[:], bm[:], mybir.AxisListType.X)
            nb = sm_pool.tile([T, 1], f32)
            nc.vector.tensor_scalar_mul(nb[:], mx[:], -SCALE)
            prob = sm_pool.tile([T, N], f32)
            se = sm_pool.tile([T, 1], f32)
            nc.scalar.activation(prob[:], bm[:],
                                 mybir.ActivationFunctionType.Exp,
                                 bias=nb[:], scale=SCALE, accum_out=se[:])
            ri = sm_pool.tile([T, 1], f32)
            nc.vector.reciprocal(ri[:], se[:])
            att_sb = sm_pool.tile([T, N], f32)
            nc.vector.tensor_scalar_mul(att_sb[:], prob[:], ri[:])
            m8 = sm_pool.tile([T, TOPK], f32)
            nc.vector.max(out=m8[:], in_=att_sb[:])
            ix = sm_pool.tile([T, TOPK], mybir.dt.uint32)
            nc.vector.max_index(ix[:], m8[:], att_sb[:])
            nc.sync.dma_start(att_d[b], att_sb[:])
            nc.sync.dma_start(idx_d[b], ix[:])
    nc.compile()
    return nc


def _get_prog():
    global _PROG
    if _PROG is None:
        _PROG = _build_program()
    return _PROG


# test.py can set this to collect profiling info; the last BassKernelResults
# is stashed in LAST_RESULT.
TRACE = bool(int(os.environ.get("BASS_KERNEL_TRACE", "0")))
LAST_RESULT = None


def _make_in_map(query, keys, i):
    """Host-side shard + relayout for core i (batches 2i, 2i+1)."""
    # keys (N, B, L*H) -> [b, m, p, h]: m = n//2, p = (n%2)*64 + pos
    kb = keys[:, i * BLOC:(i + 1) * BLOC, :]             # (N, 2, L*H)
    kb = kb.reshape(N // 2, 2, BLOC, L, H)               # (m, a, b, l, h)
    kb = np.transpose(kb, (2, 0, 1, 3, 4)).reshape(BLOC, N // 2, 128, H)
    # 32x32 block swap within each (128 pos, 128 h-chunk) matrix so the
    # device-side DVE stream-transpose (in-place 32x32 blocks) yields K^T:
    # X[b, m, 32u+i, 128c+32v+j] = kb[b, m, 32v+i, 128c+32u+j]
    v = kb.reshape(BLOC, N // 2, 4, 32, 4, 4, 32)        # b m U i C V j
    x = np.ascontiguousarray(np.transpose(v, (0, 1, 5, 3, 4, 2, 6)))
    x = x.reshape(BLOC, N // 2, 128, H)
    # qt [k, b*64 + c*16 + t] = query[t, 0, 2i+b, c*128+k]
    q2 = query[:, 0, i * BLOC:(i + 1) * BLOC, :]         # (T, 2, H)
    qt = np.ascontiguousarray(
        np.transpose(q2.reshape(T, BLOC, 4, 128), (3, 1, 2, 0))
    ).reshape(128, BLOC * 4 * T)
    return {"keys": x, "qt": qt}


def _make_runner(nc, in_maps):
    """Build a reusable jitted 8-core callable for nc (timing use)."""
    import jax
    import numpy as np_
    from jax.sharding import Mesh, PartitionSpec
    from jax.experimental.shard_map import shard_map
    import concourse.mybir as mybir
    from concourse import bass2jax

    bass2jax.install_neuronx_cc_hook()
    partition_name = (nc.partition_id_tensor.name
                      if nc.partition_id_tensor else None)
    in_names, out_names, out_avals, zero_outs = [], [], [], []
    for alloc in nc.m.functions[0].allocations:
        if not isinstance(alloc, mybir.MemoryLocationSet):
            continue
        name = alloc.memorylocations[0].name
        if alloc.kind == "ExternalInput":
            if name != partition_name:
                in_names.append(name)
        elif alloc.kind == "ExternalOutput":
            shape = tuple(alloc.tensor_shape)
            dtype = mybir.dt.np(alloc.dtype)
            out_names.append(name)
            out_avals.append(jax.core.ShapedArray(shape, dtype))
            zero_outs.append(np_.zeros(shape, dtype))
    n_params = len(in_names)
    all_names = in_names + out_names
    if partition_name is not None:
        all_names = all_names + [partition_name]

    def _body(*args):
        operands = list(args)
        if partition_name is not None:
            operands.append(bass2jax.partition_id_tensor())
        outs = bass2jax._bass_exec_p.bind(
            *operands, out_avals=tuple(out_avals), in_names=tuple(all_names),
            out_names=tuple(out_names), lowering_input_output_aliases=(),
            sim_require_finite=True, sim_require_nnan=True, nc=nc)
        return tuple(outs)

    n = len(in_maps)
    devices = jax.devices()[:n]
    mesh = Mesh(np.asarray(devices), ("core",))
    nspec = (PartitionSpec("core"),) * (n_params + len(out_names))
    jitted = jax.jit(shard_map(_body, mesh=mesh, in_specs=nspec,
                               out_specs=(PartitionSpec("core"),) * len(out_names),
                               check_rep=False), keep_unused=True)
    from jax.sharding import NamedSharding
    shard = NamedSharding(mesh, PartitionSpec("core"))
    # upload once; reuse device-resident arrays across timed calls
    concat_in = [jax.device_put(
        np.concatenate([np.asarray(m[nm]) for m in in_maps], axis=0), shard)
        for nm in in_names]
    concat_zeros = [jax.device_put(
        np.zeros((n * z.shape[0], *z.shape[1:]), z.dtype), shard)
        for z in zero_outs]
    jax.block_until_ready(concat_in)

    def run():
        outs = jitted(*concat_in, *concat_zeros)
        jax.block_until_ready(outs)
        return outs

    return run


def benchmark(query, keys, r_short=9, r_long=65, iters=12):
    """Return estimated per-iteration HW time (ns) via loop-delta timing:
    the same program is run with a For_i repeat of r_short vs r_long; the
    wall-time delta divided by the iteration delta cancels all per-call
    dispatch/load overhead."""
    import time
    query = np.asarray(query, dtype=np.float32)
    keys = np.asarray(keys, dtype=np.float32)
    in_maps = [_make_in_map(query, keys, i) for i in range(NCORES)]
    results = {}
    for r in (r_short, r_long):
        nc = _build_program(repeat=r)
        run = _make_runner(nc, in_maps)
        run()  # compile + warmup
        times = []
        for _ in range(iters):
            t0 = time.perf_counter()
            run()
            times.append(time.perf_counter() - t0)
        times.sort()
        best = times[0]
        med = times[len(times) // 2]
        results[r] = (best, med)
        print(f"  repeat={r}: best {best*1e3:.3f} ms  median {med*1e3:.3f} ms")
    ns_best = (results[r_long][0] - results[r_short][0]) / (r_long - r_short) * 1e9
    ns_med = (results[r_long][1] - results[r_short][1]) / (r_long - r_short) * 1e9
    print(f"  per-iteration: best-delta {ns_best:.0f} ns  median-delta {ns_med:.0f} ns")
    return ns_med


def kernel(query, keys):
    global LAST_RESULT
    from concourse.bass_utils import run_bass_kernel_spmd

    query = np.asarray(query, dtype=np.float32)
    keys = np.asarray(keys, dtype=np.float32)
    assert query.shape == (T, 1, B, H), query.shape
    assert keys.shape == (N, B, L * H), keys.shape

    nc = _get_prog()
    in_maps = []
    for i in range(NCORES):
        in_maps.append(_make_in_map(query, keys, i))

    res = run_bass_kernel_spmd(nc, in_maps, core_ids=list(range(NCORES)),
                               trace=TRACE)
    LAST_RESULT = res

    attention = np.empty((T, B, N), np.float32)
    topk = np.empty((TOPK, T, B), np.int32)
    for i, r in enumerate(res.results):
        attention[:, i * BLOC:(i + 1) * BLOC, :] = r["att"].transpose(1, 0, 2)
        topk[:, :, i * BLOC:(i + 1) * BLOC] = \
            r["idx"].astype(np.int32).transpose(2, 1, 0)
    return attention, topk
